# revision 1
# baseline (speedup 1.0000x reference)
"""Multi-head latent attention (MLA) Bass kernel for 8 TRN2 NeuronCores.

Sharding: tensor-parallel over heads x data-parallel over batch.
Core c (0..7) owns batch b = c//4 and head group g = c%4 (8 heads of 32).
Each core computes, for its batch:
    latentT = (hs @ Wc)^T          (replicated within the 4-core batch group)
    qT_h, kT_h (RoPE'd, transposed [head_dim, seq]) and v for its 8 heads
    attention with transposed scores [s_k, s_q] (softmax normalizer via a
    ones-matmul on PE; no max-subtraction -- scores are O(1) by construction)
    partial_out = attn(8 heads) @ Wo[rows of its heads]   -> [S, D] fp32
Host sums the 4 partials per batch. No cross-core collectives.

Compute dtype: bf16 on the TensorE inputs, fp32 PSUM accumulation.
Validated numerically: ~0.6% absmax-relative error vs the fp32 reference.
"""

import sys

for _p in ("/opt/trn_rl_repo", "/root/.axon_site/_ro/trn_rl_repo"):
    if _p not in sys.path:
        sys.path.insert(0, _p)

import numpy as np
import ml_dtypes

import concourse.bacc as bacc
import concourse.mybir as mybir
import concourse.tile as tile
from concourse import bass_isa
from concourse.bass_utils import run_bass_kernel_spmd

BF = mybir.dt.bfloat16
F32 = mybir.dt.float32
BF_NP = ml_dtypes.bfloat16

# Full-problem constants (hardcoded per the self-contained-kernel contract).
D_MODEL = 4096
D_LATENT = 512
NUM_HEADS = 32
HEAD_DIM = 128
ROPE_THETA = 10000.0
BATCH, SEQ = 2, 2048
N_CORES = 8
HEADS_PER_CORE = NUM_HEADS // 4  # 4 head groups x 2 batches = 8 cores


def build_nc(S=SEQ, D=D_MODEL, L=D_LATENT, H=HEADS_PER_CORE, Dh=HEAD_DIM,
             NA=256, NC=512):
    """Build the single-core Bass program (SPMD across 8 cores)."""
    assert S % NA == 0 and S % 128 == 0 and D % 128 == 0 and L % 128 == 0
    NC = min(NC, S)
    KD = D // 128     # contraction chunks over d_model
    LD = L // 128     # contraction chunks over d_latent
    JA = S // NA      # seq chunks in projection phase
    JC = S // NC      # seq chunks in attention phase
    SK = S // 128     # key-position chunks
    ST = S // 128     # seq tiles for the output projection
    HD1 = H * Dh      # this core's total head width (1024)
    ND = D // NC      # output-column chunks

    nc = bacc.Bacc("TRN2", target_bir_lowering=False)

    hsT_d = nc.declare_dram_parameter("hsT", [D, S], BF, isOutput=False)
    wq_d = nc.declare_dram_parameter("Wq", [D, HD1], BF, isOutput=False)
    wc_d = nc.declare_dram_parameter("Wc", [D, L], BF, isOutput=False)
    wk_d = nc.declare_dram_parameter("Wk", [L, HD1], BF, isOutput=False)
    wv_d = nc.declare_dram_parameter("Wv", [L, HD1], BF, isOutput=False)
    wo_d = nc.declare_dram_parameter("Wo", [HD1, D], BF, isOutput=False)
    cosq_d = nc.declare_dram_parameter("cosq", [Dh, S], BF, isOutput=False)
    sinq_d = nc.declare_dram_parameter("sinq", [Dh, S], BF, isOutput=False)
    cosk_d = nc.declare_dram_parameter("cosk", [Dh, S], BF, isOutput=False)
    sink_d = nc.declare_dram_parameter("sink", [Dh, S], BF, isOutput=False)
    out_d = nc.declare_dram_parameter("out", [S, D], F32, isOutput=True)
    SQ = S // 4  # this core's latent shard width (batch group of 4 cores)
    hsl_d = nc.declare_dram_parameter("hsL", [D, SQ], BF, isOutput=False)
    latq_d = nc.dram_tensor("latq_dram", [L, SQ], BF)
    latg_d = nc.dram_tensor("latg_dram", [4 * L, SQ], BF)

    # DRAM bounce for attention outputs between phases C and D (keeps the
    # SBUF pool lifetimes properly LIFO-nested). One tensor per (head,
    # seq-chunk) so phase D's reads only depend on the chunks they touch.
    attn_d = [[nc.dram_tensor(f"attnT_dram_{h}_{jc}", [Dh, min(NC, S)], BF)
               for jc in range(S // min(NC, S))] for h in range(H)]

    Exp = mybir.ActivationFunctionType.Exp
    half = Dh // 2

    with tile.TileContext(nc) as tc:
        with tc.tile_pool(name="consts", bufs=1) as const_pool:
            ones_sk = const_pool.tile([128, 128], BF)
            nc.vector.memset(ones_sk[:], 1.0)
            with tc.tile_pool(name="qT", bufs=1) as qT_pool, \
                 tc.tile_pool(name="latentT", bufs=1) as latent_pool:
                qT_t = [qT_pool.tile([Dh, S], BF, name=f"qT{h}") for h in range(H)]
                latentT_t = [latent_pool.tile([128, S], BF, name=f"latentT{ld}")
                             for ld in range(LD)]

                # ========== Phase A: latentT + qT (with RoPE) ==========
                with tc.tile_pool(name="wqA", bufs=1) as wqA_pool, \
                     tc.tile_pool(name="wcA", bufs=1) as wcA_pool, \
                     tc.tile_pool(name="hsA", bufs=KD + 4) as hsA_pool, \
                     tc.tile_pool(name="ropeq", bufs=1) as ropeq_pool, \
                     tc.tile_pool(name="tmpA", bufs=4) as tmpA_pool, \
                     tc.tile_pool(name="psA", bufs=4, space="PSUM") as psA_pool:

                    wq_t = [wqA_pool.tile([128, HD1], BF, name=f"wq{kd}")
                            for kd in range(KD)]
                    wc_t = [wcA_pool.tile([128, L], BF, name=f"wc{kd}")
                            for kd in range(KD)]
                    for kd in range(KD):
                        nc.sync.dma_start(
                            out=wc_t[kd][:], in_=wc_d[kd * 128:(kd + 1) * 128, :])
                    cosq_sb = ropeq_pool.tile([Dh, S], BF)
                    sinq_sb = ropeq_pool.tile([Dh, S], BF)

                    # --- latent shard (1/4 of seq) + AllGather across the
                    # 4-core batch group; hidden behind the qT loop below ---
                    NL = min(NA, SQ)
                    with tc.tile_pool(name="lq", bufs=4) as lq_pool, \
                         tc.tile_pool(name="hsl", bufs=KD + 2) as hsl_pool:
                        for jq in range(SQ // NL):
                            qq = slice(jq * NL, (jq + 1) * NL)
                            hl_ch = []
                            for kd in range(KD):
                                t = hsl_pool.tile([128, NL], BF, tag="hsl",
                                                  name=f"hsL_{jq}_{kd}")
                                nc.sync.dma_start(
                                    out=t[:],
                                    in_=hsl_d[kd * 128:(kd + 1) * 128, qq])
                                hl_ch.append(t)
                            for ld in range(LD):
                                ps = psA_pool.tile([128, NL], F32, tag="psA",
                                                   name=f"psLq{jq}_{ld}")
                                for kd in range(KD):
                                    nc.tensor.matmul(
                                        ps[:],
                                        wc_t[kd][:, ld * 128:(ld + 1) * 128],
                                        hl_ch[kd][:],
                                        start=(kd == 0), stop=(kd == KD - 1))
                                lq = lq_pool.tile([128, NL], BF, tag="lq",
                                                  name=f"lq{jq}_{ld}")
                                nc.scalar.copy(lq[:], ps[:])
                                nc.sync.dma_start(
                                    out=latq_d[ld * 128:(ld + 1) * 128, qq],
                                    in_=lq[:])
                    nc.gpsimd.collective_compute(
                        "AllGather",
                        mybir.AluOpType.bypass,
                        replica_groups=[[0, 1, 2, 3], [4, 5, 6, 7]],
                        ins=[latq_d[:]],
                        outs=[latg_d[:]],
                    )
                    for ld in range(LD):
                        for r in range(4):
                            nc.sync.dma_start(
                                out=latentT_t[ld][:, r * SQ:(r + 1) * SQ],
                                in_=latg_d[r * L + ld * 128:
                                           r * L + (ld + 1) * 128, :])

                    for j in range(JA):
                        jj = slice(j * NA, (j + 1) * NA)
                        hs_ch = []
                        for kd in range(KD):
                            t = hsA_pool.tile([128, NA], BF, tag="hsA",
                                              name=f"hsA_{j}_{kd}")
                            nc.sync.dma_start(
                                out=t[:], in_=hsT_d[kd * 128:(kd + 1) * 128, jj])
                            hs_ch.append(t)
                        if j == 0:
                            # weights not needed for the first (latent) groups
                            # get DMA'd after j0's activations: the first
                            # matmul only waits on wc[0] + hs[0].
                            for kd in range(KD):
                                nc.sync.dma_start(
                                    out=wq_t[kd][:],
                                    in_=wq_d[kd * 128:(kd + 1) * 128, :])
                            nc.sync.dma_start(out=cosq_sb[:], in_=cosq_d[:])
                            nc.sync.dma_start(out=sinq_sb[:], in_=sinq_d[:])
                        for h in range(H):
                            ps = psA_pool.tile([128, NA], F32, tag="psA",
                                               name=f"psQ{j}_{h}")
                            for kd in range(KD):
                                nc.tensor.matmul(
                                    ps[:], wq_t[kd][:, h * Dh:(h + 1) * Dh],
                                    hs_ch[kd][:],
                                    start=(kd == 0), stop=(kd == KD - 1))
                            t1 = tmpA_pool.tile([128, NA], F32, tag="t1",
                                                name=f"t1q{j}_{h}")
                            t2 = tmpA_pool.tile([128, NA], F32, tag="t2",
                                                name=f"t2q{j}_{h}")
                            nc.vector.tensor_mul(t1[:], ps[:], cosq_sb[:, jj])
                            nc.vector.tensor_mul(t2[0:half, :], ps[half:Dh, :],
                                                 sinq_sb[0:half, jj])
                            nc.vector.tensor_mul(t2[half:Dh, :], ps[0:half, :],
                                                 sinq_sb[half:Dh, jj])
                            nc.vector.tensor_add(qT_t[h][:, jj], t1[:], t2[:])

                # ========== Phase B: kT (with RoPE) + v ==========
                with tc.tile_pool(name="kT", bufs=1) as kT_pool, \
                     tc.tile_pool(name="v", bufs=1) as v_pool:
                    kT_t = [kT_pool.tile([Dh, S], BF, name=f"kT{h}")
                            for h in range(H)]
                    v_t = [v_pool.tile([128, HD1], BF, name=f"v{i}")
                           for i in range(SK)]

                    with tc.tile_pool(name="wkv", bufs=1) as wkv_pool, \
                         tc.tile_pool(name="ropek", bufs=1) as ropek_pool, \
                         tc.tile_pool(name="tmpB", bufs=4) as tmpB_pool, \
                         tc.tile_pool(name="psB", bufs=4, space="PSUM") as psB_pool:

                        wk_t = [wkv_pool.tile([128, HD1], BF, name=f"wk{ld}")
                                for ld in range(LD)]
                        wv_t = [wkv_pool.tile([128, HD1], BF, name=f"wv{ld}")
                                for ld in range(LD)]
                        for ld in range(LD):
                            nc.sync.dma_start(
                                out=wk_t[ld][:],
                                in_=wk_d[ld * 128:(ld + 1) * 128, :])
                            nc.sync.dma_start(
                                out=wv_t[ld][:],
                                in_=wv_d[ld * 128:(ld + 1) * 128, :])
                        cosk_sb = ropek_pool.tile([Dh, S], BF)
                        sink_sb = ropek_pool.tile([Dh, S], BF)
                        nc.sync.dma_start(out=cosk_sb[:], in_=cosk_d[:])
                        nc.sync.dma_start(out=sink_sb[:], in_=sink_d[:])

                        NB = min(512, S)
                        for h in range(H):
                            for j in range(S // NB):
                                jj = slice(j * NB, (j + 1) * NB)
                                ps = psB_pool.tile([128, NB], F32, tag="psB",
                                                   name=f"psK{h}_{j}")
                                for ld in range(LD):
                                    nc.tensor.matmul(
                                        ps[:], wk_t[ld][:, h * Dh:(h + 1) * Dh],
                                        latentT_t[ld][:, jj],
                                        start=(ld == 0), stop=(ld == LD - 1))
                                t1 = tmpB_pool.tile([128, NB], F32, tag="t1b",
                                                    name=f"t1k{h}_{j}")
                                t2 = tmpB_pool.tile([128, NB], F32, tag="t2b",
                                                    name=f"t2k{h}_{j}")
                                nc.vector.tensor_mul(t1[:], ps[:], cosk_sb[:, jj])
                                nc.vector.tensor_mul(t2[0:half, :],
                                                     ps[half:Dh, :],
                                                     sink_sb[0:half, jj])
                                nc.vector.tensor_mul(t2[half:Dh, :],
                                                     ps[0:half, :],
                                                     sink_sb[half:Dh, jj])
                                nc.vector.tensor_add(kT_t[h][:, jj], t1[:], t2[:])

                        NV = min(512, HD1)
                        for i in range(SK):
                            for cch in range(HD1 // NV):
                                cc = slice(cch * NV, (cch + 1) * NV)
                                ps = psB_pool.tile([128, NV], F32, tag="psB",
                                                   name=f"psV{i}_{cch}")
                                for ld in range(LD):
                                    nc.tensor.matmul(
                                        ps[:],
                                        latentT_t[ld][:, i * 128:(i + 1) * 128],
                                        wv_t[ld][:, cc],
                                        start=(ld == 0), stop=(ld == LD - 1))
                                nc.scalar.copy(v_t[i][:, cc], ps[:])
                    # ----- Phase C: attention (jc outer, h inner) -----
                    if True:
                        with tc.tile_pool(name="ET", bufs=8) as et_pool, \
                             tc.tile_pool(name="rinv", bufs=2) as rinv_pool, \
                             tc.tile_pool(name="atst", bufs=6) as atst_pool, \
                             tc.tile_pool(name="pssc", bufs=2, space="PSUM") as pssc_pool, \
                             tc.tile_pool(name="pspv", bufs=2, space="PSUM") as pspv_pool, \
                             tc.tile_pool(name="psr", bufs=2, space="PSUM") as psr_pool:

                            assert SK % 2 == 0
                            for jc in range(JC):
                                jj = slice(jc * NC, (jc + 1) * NC)
                                for h in range(H):
                                    ets = []
                                    for i2 in range(SK // 2):
                                        ps2 = pssc_pool.tile(
                                            [128, 2 * NC], F32, tag="sc",
                                            name=f"sc{h}_{jc}_{i2}")
                                        for p in range(2):
                                            i = i2 * 2 + p
                                            nc.tensor.matmul(
                                                ps2[:, p * NC:(p + 1) * NC],
                                                kT_t[h][:, i * 128:(i + 1) * 128],
                                                qT_t[h][:, jj],
                                                start=True, stop=True)
                                        et = et_pool.tile([128, 2 * NC], BF,
                                                          tag="ET",
                                                          name=f"et{h}_{jc}_{i2}")
                                        nc.scalar.activation(et[:], ps2[:], Exp)
                                        ets.append(et)
                                    pv = pspv_pool.tile([Dh, NC], F32, tag="pv",
                                                        name=f"pv{h}_{jc}")
                                    rr = psr_pool.tile([128, NC], F32, tag="rr",
                                                       name=f"rr{h}_{jc}")
                                    for i2 in range(SK // 2):
                                        for p in range(2):
                                            i = i2 * 2 + p
                                            sl = ets[i2][:, p * NC:(p + 1) * NC]
                                            nc.tensor.matmul(
                                                pv[:],
                                                v_t[i][:, h * Dh:(h + 1) * Dh],
                                                sl, start=(i == 0),
                                                stop=(i == SK - 1))
                                    for i2 in range(SK // 2):
                                        for p in range(2):
                                            i = i2 * 2 + p
                                            sl = ets[i2][:, p * NC:(p + 1) * NC]
                                            nc.tensor.matmul(
                                                rr[:], ones_sk[:], sl,
                                                start=(i == 0),
                                                stop=(i == SK - 1))
                                    rbs = rinv_pool.tile([128, NC], F32,
                                                         tag="rbs",
                                                         name=f"rbs{h}_{jc}")
                                    nc.vector.reciprocal_approx_fast(
                                        rbs[:], rr[:])
                                    ats = atst_pool.tile([Dh, NC], BF,
                                                         tag="atst",
                                                         name=f"atst{h}_{jc}")
                                    nc.vector.tensor_mul(ats[:], pv[:], rbs[:])
                                    nc.sync.dma_start(out=attn_d[h][jc][:],
                                                      in_=ats[:])

                        # ----- Phase D: output projection (t outer) -----
                        with tc.tile_pool(name="wo", bufs=1) as wo_pool, \
                             tc.tile_pool(name="atD", bufs=2 * H + 8) as atD_pool, \
                             tc.tile_pool(name="outst", bufs=6) as outst_pool, \
                             tc.tile_pool(name="psD", bufs=6, space="PSUM") as psD_pool:
                            wo_t = [wo_pool.tile([128, D], BF, name=f"wo{h}")
                                    for h in range(H)]
                            for h in range(H):
                                nc.sync.dma_start(
                                    out=wo_t[h][:],
                                    in_=wo_d[h * 128:(h + 1) * 128, :])
                            for t in range(ST):
                                tt = slice(t * 128, (t + 1) * 128)
                                t_jc = (t * 128) // NC
                                t_off = (t * 128) % NC
                                at_t = []
                                for h in range(H):
                                    a = atD_pool.tile([Dh, 128], BF, tag="atD",
                                                      name=f"atD{t}_{h}")
                                    nc.sync.dma_start(
                                        out=a[:],
                                        in_=attn_d[h][t_jc][:, t_off:t_off + 128])
                                    at_t.append(a)
                                for ncol in range(ND):
                                    cc = slice(ncol * NC, (ncol + 1) * NC)
                                    ps = psD_pool.tile([128, NC], F32, tag="psD",
                                                       name=f"psD{t}_{ncol}")
                                    for h in range(H):
                                        nc.tensor.matmul(
                                            ps[:], at_t[h][:], wo_t[h][:, cc],
                                            start=(h == 0), stop=(h == H - 1))
                                    st = outst_pool.tile([128, NC], F32,
                                                         tag="outst",
                                                         name=f"outst{t}_{ncol}")
                                    nc.scalar.copy(st[:], ps[:])
                                    nc.sync.dma_start(out=out_d[tt, cc],
                                                      in_=st[:])

    nc.compile()
    return nc


def host_inputs(hidden_states, Wq, Wc, Wk, Wv, Wo, S=SEQ, Dh=HEAD_DIM,
                heads_per_core=HEADS_PER_CORE, n_cores=N_CORES):
    """Shard + preprocess full fp32 inputs into per-core bf16 in_maps."""
    scale = 1.0 / np.sqrt(Dh)
    pos = np.arange(S, dtype=np.float32)
    inv_freq = 1.0 / (ROPE_THETA ** (np.arange(0, Dh, 2, dtype=np.float32) / Dh))
    freqs = pos[:, None] * inv_freq
    emb = np.concatenate([freqs, freqs], axis=-1)      # [S, Dh]
    cosT = np.cos(emb).T.copy()                        # [Dh, S]
    sinT = np.sin(emb).T.copy()
    sinT[: Dh // 2] *= -1.0                            # sign baked for the swap trick
    cosq = (cosT * scale).astype(BF_NP)
    sinq = (sinT * scale).astype(BF_NP)
    cosk = cosT.astype(BF_NP)
    sink = sinT.astype(BF_NP)

    hw = heads_per_core * Dh
    in_maps = []
    for c in range(n_cores):
        b, g = divmod(c, 4)
        cols = slice(g * hw, (g + 1) * hw)
        sq = S // 4
        in_maps.append({
            "hsT": np.ascontiguousarray(hidden_states[b].T).astype(BF_NP),
            "hsL": np.ascontiguousarray(
                hidden_states[b].T[:, g * sq:(g + 1) * sq]).astype(BF_NP),
            "Wq": np.ascontiguousarray(Wq[:, cols]).astype(BF_NP),
            "Wc": Wc.astype(BF_NP),
            "Wk": np.ascontiguousarray(Wk[:, cols]).astype(BF_NP),
            "Wv": np.ascontiguousarray(Wv[:, cols]).astype(BF_NP),
            "Wo": np.ascontiguousarray(Wo[cols, :]).astype(BF_NP),
            "cosq": cosq, "sinq": sinq, "cosk": cosk, "sink": sink,
        })
    return in_maps


_NC_CACHE = {}


def kernel(hidden_states, Wq, Wc, Wk, Wv, Wo):
    hidden_states = np.asarray(hidden_states, dtype=np.float32)
    if "nc" not in _NC_CACHE:
        _NC_CACHE["nc"] = build_nc()
    nc = _NC_CACHE["nc"]
    in_maps = host_inputs(hidden_states, np.asarray(Wq, np.float32),
                          np.asarray(Wc, np.float32), np.asarray(Wk, np.float32),
                          np.asarray(Wv, np.float32), np.asarray(Wo, np.float32))
    res = run_bass_kernel_spmd(nc, in_maps, list(range(N_CORES))).results
    B, S, D = BATCH, SEQ, D_MODEL
    out = np.zeros((B, S, D), dtype=np.float32)
    for c in range(N_CORES):
        out[c // 4] += res[c]["out"]
    return out



# revision 2
# speedup vs baseline: 1.0560x; 1.0560x over previous
"""Multi-head latent attention (MLA) Bass kernel for 8 TRN2 NeuronCores. v2.

Sharding: tensor-parallel over heads x data-parallel over batch.
Core c (0..7) owns batch b = c//4 and head group g = c%4 (8 heads of 32).

v2 changes vs baseline:
  - softmax normalizer via DVE pairwise-sum tree (bf16) + a single
    128-wide ones-matmul, instead of a full second pass of PE matmuls
    (saves ~250k PE cycles/core).
  - attention outputs stay in SBUF (no DRAM bounce); output projection
    first half (d_model cols 0:2048) is interleaved into the attention
    loop as PE filler while ScalarE runs exp; second half runs as a
    pure-PE tail after SBUF pressure drops.
  - phase-B weights / Wo / rope tables are DMA'd early so PE never waits.
"""

import sys

for _p in ("/opt/trn_rl_repo", "/root/.axon_site/_ro/trn_rl_repo"):
    if _p not in sys.path:
        sys.path.insert(0, _p)

import numpy as np
import ml_dtypes

import concourse.bacc as bacc
import concourse.mybir as mybir
import concourse.tile as tile
from concourse.bass_utils import run_bass_kernel_spmd

BF = mybir.dt.bfloat16
F32 = mybir.dt.float32
BF_NP = ml_dtypes.bfloat16

# Full-problem constants (hardcoded per the self-contained-kernel contract).
D_MODEL = 4096
D_LATENT = 512
NUM_HEADS = 32
HEAD_DIM = 128
ROPE_THETA = 10000.0
BATCH, SEQ = 2, 2048
N_CORES = 8
HEADS_PER_CORE = NUM_HEADS // 4  # 4 head groups x 2 batches = 8 cores


def build_nc(S=SEQ, D=D_MODEL, L=D_LATENT, H=HEADS_PER_CORE, Dh=HEAD_DIM):
    NA = 512          # seq chunk in projection phase
    NC = 512          # query chunk in attention phase
    KD = D // 128     # 32 contraction chunks over d_model
    LD = L // 128     # 4 contraction chunks over d_latent
    JA = S // NA      # 4 seq chunks in projection phase
    JC = S // NC      # 4 query chunks in attention phase
    SK = S // 128     # 16 key-position chunks
    HD1 = H * Dh      # 1024, this core's head width
    SQ = S // 4       # 512, latent shard width (batch group of 4 cores)
    DH2 = D // 2      # 2048, half of model dim (o_proj column split)
    half = Dh // 2

    nc = bacc.Bacc("TRN2", target_bir_lowering=False)

    hsT_d = nc.declare_dram_parameter("hsT", [D, S], BF, isOutput=False)
    wq_d = nc.declare_dram_parameter("Wq", [D, HD1], BF, isOutput=False)
    wc_d = nc.declare_dram_parameter("Wc", [D, L], BF, isOutput=False)
    wk_d = nc.declare_dram_parameter("Wk", [L, HD1], BF, isOutput=False)
    wv_d = nc.declare_dram_parameter("Wv", [L, HD1], BF, isOutput=False)
    wo_d = nc.declare_dram_parameter("Wo", [HD1, D], BF, isOutput=False)
    cosq_d = nc.declare_dram_parameter("cosq", [Dh, S], BF, isOutput=False)
    sinq_d = nc.declare_dram_parameter("sinq", [Dh, S], BF, isOutput=False)
    cosk_d = nc.declare_dram_parameter("cosk", [Dh, S], BF, isOutput=False)
    sink_d = nc.declare_dram_parameter("sink", [Dh, S], BF, isOutput=False)
    out_d = nc.declare_dram_parameter("out", [S, D], F32, isOutput=True)
    latq_d = nc.dram_tensor("latq_dram", [L, SQ], BF)
    latg_d = nc.dram_tensor("latg_dram", [4 * L, SQ], BF)

    Exp = mybir.ActivationFunctionType.Exp

    with tile.TileContext(nc) as tc:
        with tc.tile_pool(name="consts", bufs=1) as const_pool:
            ones_sk = const_pool.tile([128, 128], BF)
            nc.vector.memset(ones_sk[:], 1.0)
            with tc.tile_pool(name="qT", bufs=1) as qT_pool:
                qT_t = [qT_pool.tile([Dh, S], BF, name=f"qT{h}") for h in range(H)]

                # Each core's hsT is ROTATED host-side by -g*SQ columns so
                # its latent shard is seq chunk 0: the latent pass and
                # q-proj j=0 share the same hs tiles (no separate hsL DMA).
                # cosq/sinq are rotated to match; the host un-rotates the
                # output rows.
                with tc.tile_pool(name="hsA", bufs=KD + 6) as hsA_pool:
                    # ===== Phase A0: latent shard + AllGather =====
                    hs0_ch = []
                    with nc.named_scope("latent"), \
                         tc.tile_pool(name="wcA", bufs=1) as wcA_pool, \
                         tc.tile_pool(name="lq", bufs=4) as lq_pool, \
                         tc.tile_pool(name="psL", bufs=4, space="PSUM") as psL_pool:
                        wc_t = []
                        for kd in range(KD):
                            w = wcA_pool.tile([128, L], BF, name=f"wc{kd}")
                            nc.sync.dma_start(
                                out=w[:], in_=wc_d[kd * 128:(kd + 1) * 128, :])
                            t = hsA_pool.tile([128, NA], BF, tag="hsA",
                                              name=f"hsA_0_{kd}")
                            nc.sync.dma_start(
                                out=t[:],
                                in_=hsT_d[kd * 128:(kd + 1) * 128, 0:NA])
                            wc_t.append(w)
                            hs0_ch.append(t)
                        # kd-outer accumulation into 4 concurrent PSUM banks
                        # so the matmul stream pipelines with the DMA stream
                        psL_t = [psL_pool.tile([128, SQ], F32, tag="psL",
                                               name=f"psLq{ld}")
                                 for ld in range(LD)]
                        for kd in range(KD):
                            for ld in range(LD):
                                nc.tensor.matmul(
                                    psL_t[ld][:],
                                    wc_t[kd][:, ld * 128:(ld + 1) * 128],
                                    hs0_ch[kd][:],
                                    start=(kd == 0), stop=(kd == KD - 1))
                        for ld in range(LD):
                            lq = lq_pool.tile([128, SQ], BF, tag="lq",
                                              name=f"lq{ld}")
                            nc.scalar.copy(lq[:], psL_t[ld][:])
                            nc.sync.dma_start(
                                out=latq_d[ld * 128:(ld + 1) * 128, :],
                                in_=lq[:])
                    nc.gpsimd.collective_compute(
                        "AllGather",
                        mybir.AluOpType.bypass,
                        replica_groups=[[0, 1, 2, 3], [4, 5, 6, 7]],
                        ins=[latq_d[:]],
                        outs=[latg_d[:]],
                    )

                    # ===== Phase A: qT = RoPE(hs @ Wq)^T =====
                    with nc.named_scope("qproj"), \
                         tc.tile_pool(name="wqA", bufs=1) as wqA_pool, \
                         tc.tile_pool(name="ropeq", bufs=4) as ropeq_pool, \
                         tc.tile_pool(name="tmpA", bufs=4) as tmpA_pool, \
                         tc.tile_pool(name="psA", bufs=4, space="PSUM") as psA_pool:
                        # Wq streamed in column quarters (2 heads each) so the
                        # first q-proj matmuls can start while later quarters
                        # are still in flight
                        wq_t = [wqA_pool.tile([128, HD1], BF, name=f"wq{kd}")
                                for kd in range(KD)]
                        QW = HD1 // 4
                        for q4 in range(4):
                            for kd in range(KD):
                                nc.sync.dma_start(
                                    out=wq_t[kd][:, q4 * QW:(q4 + 1) * QW],
                                    in_=wq_d[kd * 128:(kd + 1) * 128,
                                             q4 * QW:(q4 + 1) * QW])

                        for j in range(JA):
                            jj = slice(j * NA, (j + 1) * NA)
                            if j == 0:
                                hs_ch = hs0_ch
                            else:
                                hs_ch = []
                                for kd in range(KD):
                                    t = hsA_pool.tile([128, NA], BF, tag="hsA",
                                                      name=f"hsA_{j}_{kd}")
                                    nc.sync.dma_start(
                                        out=t[:],
                                        in_=hsT_d[kd * 128:(kd + 1) * 128, jj])
                                    hs_ch.append(t)
                            cq = ropeq_pool.tile([Dh, NA], BF, tag="cq",
                                                 name=f"cq{j}")
                            sq = ropeq_pool.tile([Dh, NA], BF, tag="sq",
                                                 name=f"sq{j}")
                            nc.sync.dma_start(out=cq[:], in_=cosq_d[:, jj])
                            nc.sync.dma_start(out=sq[:], in_=sinq_d[:, jj])
                            for h in range(H):
                                ps = psA_pool.tile([128, NA], F32, tag="psA",
                                                   name=f"psQ{j}_{h}")
                                for kd in range(KD):
                                    nc.tensor.matmul(
                                        ps[:], wq_t[kd][:, h * Dh:(h + 1) * Dh],
                                        hs_ch[kd][:],
                                        start=(kd == 0), stop=(kd == KD - 1))
                                t1 = tmpA_pool.tile([128, NA], F32, tag="t1",
                                                    name=f"t1q{j}_{h}")
                                t2 = tmpA_pool.tile([128, NA], F32, tag="t2",
                                                    name=f"t2q{j}_{h}")
                                nc.vector.tensor_mul(t1[:], ps[:], cq[:])
                                nc.vector.tensor_mul(t2[0:half, :],
                                                     ps[half:Dh, :],
                                                     sq[0:half, :])
                                nc.vector.tensor_mul(t2[half:Dh, :],
                                                     ps[0:half, :],
                                                     sq[half:Dh, :])
                                nc.vector.tensor_add(qT_t[h][:, jj],
                                                     t1[:], t2[:])

                # ========== outer pools for attention + o_proj ==========
                with tc.tile_pool(name="ats", bufs=1) as ats_pool, \
                     tc.tile_pool(name="wo1", bufs=1) as wo1_pool:
                    ats_t = [[ats_pool.tile([Dh, NC], BF, name=f"ats{h}_{jc}")
                              for jc in range(JC)] for h in range(H)]
                    wo1_t = []
                    for h in range(H):
                        w = wo1_pool.tile([128, DH2], BF, name=f"wo1_{h}")
                        nc.sync.dma_start(
                            out=w[:], in_=wo_d[h * 128:(h + 1) * 128, 0:DH2])
                        wo1_t.append(w)

                    with tc.tile_pool(name="kT", bufs=1) as kT_pool, \
                         tc.tile_pool(name="v", bufs=1) as v_pool:
                        kT_t = [kT_pool.tile([Dh, S], BF, name=f"kT{h}")
                                for h in range(H)]
                        v_t = [v_pool.tile([128, HD1], BF, name=f"v{i}")
                               for i in range(SK)]

                        # ========== Phase B: kT (RoPE, bf16 path) + v =====
                        with nc.named_scope("kv"), \
                             tc.tile_pool(name="latB", bufs=1) as latB_pool:
                            lat_t = [latB_pool.tile([128, S], BF,
                                                    name=f"latB{ld}")
                                     for ld in range(LD)]
                            for ld in range(LD):
                                for r in range(4):
                                    nc.sync.dma_start(
                                        out=lat_t[ld][:, r * SQ:(r + 1) * SQ],
                                        in_=latg_d[r * L + ld * 128:
                                                   r * L + (ld + 1) * 128, :])
                            with tc.tile_pool(name="wkv", bufs=1) as wkv_pool, \
                                 tc.tile_pool(name="ropek", bufs=1) as ropek_pool, \
                                 tc.tile_pool(name="tmpB", bufs=6) as tmpB_pool, \
                                 tc.tile_pool(name="psB", bufs=4, space="PSUM") as psB_pool, \
                                 tc.tile_pool(name="psV", bufs=4, space="PSUM") as psV_pool:
                                wk_t = []
                                wv_t = []
                                for ld in range(LD):
                                    w = wkv_pool.tile([128, HD1], BF,
                                                      name=f"wk{ld}")
                                    nc.sync.dma_start(
                                        out=w[:],
                                        in_=wk_d[ld * 128:(ld + 1) * 128, :])
                                    wk_t.append(w)
                                    w = wkv_pool.tile([128, HD1], BF,
                                                      name=f"wv{ld}")
                                    nc.sync.dma_start(
                                        out=w[:],
                                        in_=wv_d[ld * 128:(ld + 1) * 128, :])
                                    wv_t.append(w)
                                cosk_sb = ropek_pool.tile([Dh, S], BF)
                                sink_sb = ropek_pool.tile([Dh, S], BF)
                                nc.sync.dma_start(out=cosk_sb[:], in_=cosk_d[:])
                                nc.sync.dma_start(out=sink_sb[:], in_=sink_d[:])

                                # v-groups interleaved 1:1 into the kT loop:
                                # the kT RoPE is DVE-bound, the v matmuls are
                                # pure PE + ScalarE, so together they balance
                                vjobs = [(i, cch) for i in range(SK)
                                         for cch in range(2)]
                                for h in range(H):
                                    for j in range(S // NA):
                                        jj = slice(j * NA, (j + 1) * NA)
                                        ps = psB_pool.tile(
                                            [128, NA], F32, tag="psB",
                                            name=f"psK{h}_{j}")
                                        for ld in range(LD):
                                            nc.tensor.matmul(
                                                ps[:],
                                                wk_t[ld][:, h * Dh:(h + 1) * Dh],
                                                lat_t[ld][:, jj],
                                                start=(ld == 0),
                                                stop=(ld == LD - 1))
                                        # v group as PE filler
                                        vi, cch = vjobs[h * (S // NA) + j]
                                        cc = slice(cch * NA, (cch + 1) * NA)
                                        psv = psV_pool.tile(
                                            [128, NA], F32, tag="psV",
                                            name=f"psV{vi}_{cch}")
                                        for ld in range(LD):
                                            nc.tensor.matmul(
                                                psv[:],
                                                lat_t[ld][:,
                                                          vi * 128:(vi + 1) * 128],
                                                wv_t[ld][:, cc],
                                                start=(ld == 0),
                                                stop=(ld == LD - 1))
                                        nc.scalar.copy(v_t[vi][:, cc], psv[:])
                                        # kT RoPE: rotate-half reads come from
                                        # PSUM (mixed PSUM+SBUF may differ in
                                        # base partition; SBUF+SBUF may not)
                                        t1 = tmpB_pool.tile([128, NA], BF,
                                                            tag="tb",
                                                            name=f"t1k{h}_{j}")
                                        t2 = tmpB_pool.tile([128, NA], BF,
                                                            tag="tb",
                                                            name=f"t2k{h}_{j}")
                                        nc.vector.tensor_mul(
                                            t1[:], ps[:], cosk_sb[:, jj])
                                        nc.vector.tensor_mul(
                                            t2[0:half, :], ps[half:Dh, :],
                                            sink_sb[0:half, jj])
                                        nc.vector.tensor_mul(
                                            t2[half:Dh, :], ps[0:half, :],
                                            sink_sb[half:Dh, jj])
                                        nc.vector.tensor_add(
                                            kT_t[h][:, jj], t1[:], t2[:])

                        # ===== Phase C: attention, o_proj half interleaved ==
                        with nc.named_scope("attn"), \
                             tc.tile_pool(name="ET", bufs=6) as et_pool, \
                             tc.tile_pool(name="tree", bufs=8) as tree_pool, \
                             tc.tile_pool(name="rinv", bufs=2) as rinv_pool, \
                             tc.tile_pool(name="outst", bufs=4) as outst_pool, \
                             tc.tile_pool(name="psBT", bufs=2, space="PSUM") as psBT_pool, \
                             tc.tile_pool(name="psPV", bufs=2, space="PSUM") as psPV_pool, \
                             tc.tile_pool(name="psD", bufs=1, space="PSUM") as psD_pool, \
                             tc.tile_pool(name="psR", bufs=1, space="PSUM") as psR_pool:

                            d_pending = []

                            def emit_d_group(wo_tiles, col0):
                                jcp, tl, ncol = d_pending.pop(0)
                                t_global = jcp * (NC // 128) + tl
                                rows = slice(t_global * 128,
                                             (t_global + 1) * 128)
                                cc = slice(ncol * 512, (ncol + 1) * 512)
                                ps = psD_pool.tile(
                                    [128, 512], F32, tag="psD",
                                    name=f"psD{jcp}_{tl}_{ncol}")
                                for h2 in range(H):
                                    nc.tensor.matmul(
                                        ps[:],
                                        ats_t[h2][jcp][:,
                                                       tl * 128:(tl + 1) * 128],
                                        wo_tiles[h2][:, cc],
                                        start=(h2 == 0), stop=(h2 == H - 1))
                                st = outst_pool.tile(
                                    [128, 512], F32, tag="outst",
                                    name=f"st{jcp}_{tl}_{ncol}")
                                nc.vector.tensor_copy(st[:], ps[:])
                                nc.sync.dma_start(
                                    out=out_d[rows,
                                              col0 + ncol * 512:
                                              col0 + (ncol + 1) * 512],
                                    in_=st[:])

                            for jc in range(JC):
                                jj = slice(jc * NC, (jc + 1) * NC)
                                for h in range(H):
                                    ets = []
                                    sums = []   # pairwise tree over et tiles
                                    pv = psPV_pool.tile(
                                        [Dh, NC], F32, tag="pv",
                                        name=f"pv{h}_{jc}")
                                    for g in range(SK // 2):
                                        bt = psBT_pool.tile(
                                            [128, 1024], F32, tag="bt",
                                            name=f"bt{h}_{jc}_{g}")
                                        for p in range(2):
                                            i = g * 2 + p
                                            nc.tensor.matmul(
                                                bt[:, p * 512:(p + 1) * 512],
                                                kT_t[h][:, i * 128:(i + 1) * 128],
                                                qT_t[h][:, jj],
                                                start=True, stop=True)
                                        et = et_pool.tile(
                                            [128, 1024], BF, tag="ET",
                                            name=f"et{h}_{jc}_{g}")
                                        nc.scalar.activation(et[:], bt[:], Exp)
                                        ets.append(et)
                                        if g % 2 == 1:
                                            w = tree_pool.tile(
                                                [128, 1024], BF, tag="tr",
                                                name=f"s{h}_{jc}_{g}")
                                            nc.vector.tensor_add(
                                                w[:], ets[g - 1][:], ets[g][:])
                                            sums.append(w)
                                        if g in (2, 5) and d_pending:
                                            emit_d_group(wo1_t, 0)
                                        if g >= 1:
                                            for p in range(2):
                                                i = (g - 1) * 2 + p
                                                nc.tensor.matmul(
                                                    pv[:],
                                                    v_t[i][:, h * Dh:(h + 1) * Dh],
                                                    ets[g - 1][:,
                                                               p * 512:
                                                               (p + 1) * 512],
                                                    start=(i == 0),
                                                    stop=(i == SK - 1))
                                    for p in range(2):
                                        i = (SK // 2 - 1) * 2 + p
                                        nc.tensor.matmul(
                                            pv[:],
                                            v_t[i][:, h * Dh:(h + 1) * Dh],
                                            ets[SK // 2 - 1][:,
                                                             p * 512:
                                                             (p + 1) * 512],
                                            start=(i == 0), stop=(i == SK - 1))
                                    # finish tree: 4 -> 2 -> 1 wide, then fold
                                    while len(sums) > 1:
                                        nxt = []
                                        for a in range(0, len(sums), 2):
                                            w = tree_pool.tile(
                                                [128, 1024], BF, tag="tr",
                                                name=f"w{h}_{jc}_{len(sums)}_{a}")
                                            nc.vector.tensor_add(
                                                w[:], sums[a][:], sums[a + 1][:])
                                            nxt.append(w)
                                        sums = nxt
                                    rfold = tree_pool.tile(
                                        [128, NC], BF, tag="trf", bufs=2,
                                        name=f"rf{h}_{jc}")
                                    nc.vector.tensor_add(
                                        rfold[:], sums[0][:, 0:512],
                                        sums[0][:, 512:1024])
                                    rr = psR_pool.tile([128, NC], F32, tag="rr",
                                                       name=f"rr{h}_{jc}")
                                    nc.tensor.matmul(rr[:], ones_sk[:],
                                                     rfold[:],
                                                     start=True, stop=True)
                                    rbs = rinv_pool.tile([128, NC], F32,
                                                         tag="rbs",
                                                         name=f"rbs{h}_{jc}")
                                    nc.vector.reciprocal_approx_fast(
                                        rbs[:], rr[:])
                                    nc.vector.tensor_mul(
                                        ats_t[h][jc][:], pv[:], rbs[:])
                                # queue o_proj first-half for this jc
                                for tl in range(NC // 128):
                                    for ncol in range(DH2 // 512):
                                        d_pending.append((jc, tl, ncol))
                                # drain backlog from older jc chunks so the
                                # queue never exceeds one jc worth of groups
                                while len(d_pending) > 16:
                                    emit_d_group(wo1_t, 0)
                            with nc.named_scope("oproj1"):
                                while d_pending:
                                    emit_d_group(wo1_t, 0)

                    # ========== Tail: o_proj second half ==========
                    with nc.named_scope("oproj2"), \
                         tc.tile_pool(name="wo2", bufs=1) as wo2_pool, \
                         tc.tile_pool(name="outT", bufs=4) as outT_pool, \
                         tc.tile_pool(name="psT", bufs=4, space="PSUM") as psT_pool:
                        wo2_t = []
                        for h in range(H):
                            w = wo2_pool.tile([128, DH2], BF, name=f"wo2_{h}")
                            nc.sync.dma_start(
                                out=w[:], in_=wo_d[h * 128:(h + 1) * 128,
                                                   DH2:D])
                            wo2_t.append(w)
                        for t in range(S // 128):
                            rows = slice(t * 128, (t + 1) * 128)
                            jcp, tl = divmod(t, NC // 128)
                            for ncol in range(DH2 // 512):
                                cc = slice(ncol * 512, (ncol + 1) * 512)
                                ps = psT_pool.tile([128, 512], F32, tag="psT",
                                                   name=f"psT{t}_{ncol}")
                                for h in range(H):
                                    nc.tensor.matmul(
                                        ps[:],
                                        ats_t[h][jcp][:, tl * 128:(tl + 1) * 128],
                                        wo2_t[h][:, cc],
                                        start=(h == 0), stop=(h == H - 1))
                                st = outT_pool.tile([128, 512], F32, tag="oT",
                                                    name=f"oT{t}_{ncol}")
                                nc.vector.tensor_copy(st[:], ps[:])
                                nc.sync.dma_start(
                                    out=out_d[rows, DH2 + ncol * 512:
                                              DH2 + (ncol + 1) * 512],
                                    in_=st[:])

    nc.compile()
    return nc


def host_inputs(hidden_states, Wq, Wc, Wk, Wv, Wo, S=SEQ, Dh=HEAD_DIM,
                heads_per_core=HEADS_PER_CORE, n_cores=N_CORES):
    """Shard + preprocess full fp32 inputs into per-core bf16 in_maps."""
    scale = 1.0 / np.sqrt(Dh)
    pos = np.arange(S, dtype=np.float32)
    inv_freq = 1.0 / (ROPE_THETA ** (np.arange(0, Dh, 2, dtype=np.float32) / Dh))
    freqs = pos[:, None] * inv_freq
    emb = np.concatenate([freqs, freqs], axis=-1)      # [S, Dh]
    cosT = np.cos(emb).T.copy()                        # [Dh, S]
    sinT = np.sin(emb).T.copy()
    sinT[: Dh // 2] *= -1.0                            # sign baked for the swap trick
    cosq = (cosT * scale).astype(BF_NP)
    sinq = (sinT * scale).astype(BF_NP)
    cosk = cosT.astype(BF_NP)
    sink = sinT.astype(BF_NP)

    hw = heads_per_core * Dh
    in_maps = []
    for c in range(n_cores):
        b, g = divmod(c, 4)
        cols = slice(g * hw, (g + 1) * hw)
        sq = S // 4
        # rotate the query-side seq axis by -g*sq so this core's latent
        # shard is seq chunk 0 (the host un-rotates the output rows)
        rot = -g * sq
        hsT = np.roll(hidden_states[b].T, rot, axis=1)
        in_maps.append({
            "hsT": np.ascontiguousarray(hsT).astype(BF_NP),
            "Wq": np.ascontiguousarray(Wq[:, cols]).astype(BF_NP),
            "Wc": Wc.astype(BF_NP),
            "Wk": np.ascontiguousarray(Wk[:, cols]).astype(BF_NP),
            "Wv": np.ascontiguousarray(Wv[:, cols]).astype(BF_NP),
            "Wo": np.ascontiguousarray(Wo[cols, :]).astype(BF_NP),
            "cosq": np.ascontiguousarray(np.roll(cosq, rot, axis=1)),
            "sinq": np.ascontiguousarray(np.roll(sinq, rot, axis=1)),
            "cosk": cosk, "sink": sink,
        })
    return in_maps


_NC_CACHE = {}


def kernel(hidden_states, Wq, Wc, Wk, Wv, Wo):
    hidden_states = np.asarray(hidden_states, dtype=np.float32)
    if "nc" not in _NC_CACHE:
        _NC_CACHE["nc"] = build_nc()
    nc = _NC_CACHE["nc"]
    in_maps = host_inputs(hidden_states, np.asarray(Wq, np.float32),
                          np.asarray(Wc, np.float32), np.asarray(Wk, np.float32),
                          np.asarray(Wv, np.float32), np.asarray(Wo, np.float32))
    res = run_bass_kernel_spmd(nc, in_maps, list(range(N_CORES))).results
    B, S, D = BATCH, SEQ, D_MODEL
    out = np.zeros((B, S, D), dtype=np.float32)
    for c in range(N_CORES):
        b, g = divmod(c, 4)
        out[b] += np.roll(res[c]["out"], g * (S // 4), axis=0)
    return out


# revision 3
# speedup vs baseline: 1.0582x; 1.0020x over previous
"""Multi-head latent attention (MLA) Bass kernel for 8 TRN2 NeuronCores. v2.

Sharding: tensor-parallel over heads x data-parallel over batch.
Core c (0..7) owns batch b = c//4 and head group g = c%4 (8 heads of 32).

v2 changes vs baseline:
  - softmax normalizer via DVE pairwise-sum tree (bf16) + a single
    128-wide ones-matmul, instead of a full second pass of PE matmuls
    (saves ~250k PE cycles/core).
  - attention outputs stay in SBUF (no DRAM bounce); output projection
    first half (d_model cols 0:2048) is interleaved into the attention
    loop as PE filler while ScalarE runs exp; second half runs as a
    pure-PE tail after SBUF pressure drops.
  - phase-B weights / Wo / rope tables are DMA'd early so PE never waits.
"""

import sys

for _p in ("/opt/trn_rl_repo", "/root/.axon_site/_ro/trn_rl_repo"):
    if _p not in sys.path:
        sys.path.insert(0, _p)

import numpy as np
import ml_dtypes

import concourse.bacc as bacc
import concourse.mybir as mybir
import concourse.tile as tile
from concourse.bass_utils import run_bass_kernel_spmd

BF = mybir.dt.bfloat16
F32 = mybir.dt.float32
BF_NP = ml_dtypes.bfloat16

# Full-problem constants (hardcoded per the self-contained-kernel contract).
D_MODEL = 4096
D_LATENT = 512
NUM_HEADS = 32
HEAD_DIM = 128
ROPE_THETA = 10000.0
BATCH, SEQ = 2, 2048
N_CORES = 8
HEADS_PER_CORE = NUM_HEADS // 4  # 4 head groups x 2 batches = 8 cores


def build_nc(S=SEQ, D=D_MODEL, L=D_LATENT, H=HEADS_PER_CORE, Dh=HEAD_DIM):
    NA = 512          # seq chunk in projection phase
    NC = 512          # query chunk in attention phase
    KD = D // 128     # 32 contraction chunks over d_model
    LD = L // 128     # 4 contraction chunks over d_latent
    JA = S // NA      # 4 seq chunks in projection phase
    JC = S // NC      # 4 query chunks in attention phase
    SK = S // 128     # 16 key-position chunks
    HD1 = H * Dh      # 1024, this core's head width
    SQ = S // 4       # 512, latent shard width (batch group of 4 cores)
    DH2 = D // 2      # 2048, half of model dim (o_proj column split)
    half = Dh // 2

    nc = bacc.Bacc("TRN2", target_bir_lowering=False)

    hsT_d = nc.declare_dram_parameter("hsT", [D, S], BF, isOutput=False)
    wq_d = nc.declare_dram_parameter("Wq", [D, HD1], BF, isOutput=False)
    wc_d = nc.declare_dram_parameter("Wc", [D, L], BF, isOutput=False)
    wk_d = nc.declare_dram_parameter("Wk", [L, HD1], BF, isOutput=False)
    wv_d = nc.declare_dram_parameter("Wv", [L, HD1], BF, isOutput=False)
    wo_d = nc.declare_dram_parameter("Wo", [HD1, D], BF, isOutput=False)
    cosq_d = nc.declare_dram_parameter("cosq", [Dh, S], BF, isOutput=False)
    sinq_d = nc.declare_dram_parameter("sinq", [Dh, S], BF, isOutput=False)
    cosk_d = nc.declare_dram_parameter("cosk", [Dh, S], BF, isOutput=False)
    sink_d = nc.declare_dram_parameter("sink", [Dh, S], BF, isOutput=False)
    out_d = nc.declare_dram_parameter("out", [S, D], F32, isOutput=True)
    latq_d = nc.dram_tensor("latq_dram", [L, SQ], BF)
    latg_d = nc.dram_tensor("latg_dram", [4 * L, SQ], BF)

    Exp = mybir.ActivationFunctionType.Exp

    with tile.TileContext(nc) as tc:
        with tc.tile_pool(name="consts", bufs=1) as const_pool:
            ones_sk = const_pool.tile([128, 128], BF)
            nc.vector.memset(ones_sk[:], 1.0)
            with tc.tile_pool(name="qT", bufs=1) as qT_pool:
                qT_t = [qT_pool.tile([Dh, S], BF, name=f"qT{h}") for h in range(H)]

                # Each core's hsT is ROTATED host-side by -g*SQ columns so
                # its latent shard is seq chunk 0: the latent pass and
                # q-proj j=0 share the same hs tiles (no separate hsL DMA).
                # cosq/sinq are rotated to match; the host un-rotates the
                # output rows.
                with tc.tile_pool(name="hsA", bufs=KD + 6) as hsA_pool:
                    # ===== Phase A0: latent shard + AllGather =====
                    hs0_ch = []
                    with nc.named_scope("latent"), \
                         tc.tile_pool(name="wcA", bufs=1) as wcA_pool, \
                         tc.tile_pool(name="lq", bufs=4) as lq_pool, \
                         tc.tile_pool(name="psL", bufs=4, space="PSUM") as psL_pool:
                        wc_t = []
                        for kd in range(KD):
                            w = wcA_pool.tile([128, L], BF, name=f"wc{kd}")
                            nc.sync.dma_start(
                                out=w[:], in_=wc_d[kd * 128:(kd + 1) * 128, :])
                            t = hsA_pool.tile([128, NA], BF, tag="hsA",
                                              name=f"hsA_0_{kd}")
                            nc.sync.dma_start(
                                out=t[:],
                                in_=hsT_d[kd * 128:(kd + 1) * 128, 0:NA])
                            wc_t.append(w)
                            hs0_ch.append(t)
                        # kd-outer accumulation into 4 concurrent PSUM banks
                        # so the matmul stream pipelines with the DMA stream
                        psL_t = [psL_pool.tile([128, SQ], F32, tag="psL",
                                               name=f"psLq{ld}")
                                 for ld in range(LD)]
                        for kd in range(KD):
                            for ld in range(LD):
                                nc.tensor.matmul(
                                    psL_t[ld][:],
                                    wc_t[kd][:, ld * 128:(ld + 1) * 128],
                                    hs0_ch[kd][:],
                                    start=(kd == 0), stop=(kd == KD - 1))
                        for ld in range(LD):
                            lq = lq_pool.tile([128, SQ], BF, tag="lq",
                                              name=f"lq{ld}")
                            nc.scalar.copy(lq[:], psL_t[ld][:])
                            nc.sync.dma_start(
                                out=latq_d[ld * 128:(ld + 1) * 128, :],
                                in_=lq[:])
                    nc.gpsimd.collective_compute(
                        "AllGather",
                        mybir.AluOpType.bypass,
                        replica_groups=[[0, 1, 2, 3], [4, 5, 6, 7]],
                        ins=[latq_d[:]],
                        outs=[latg_d[:]],
                    )

                    # ===== Phase A: qT = RoPE(hs @ Wq)^T =====
                    with nc.named_scope("qproj"), \
                         tc.tile_pool(name="wqA", bufs=1) as wqA_pool, \
                         tc.tile_pool(name="ropeq", bufs=4) as ropeq_pool, \
                         tc.tile_pool(name="tmpA", bufs=4) as tmpA_pool, \
                         tc.tile_pool(name="psA", bufs=4, space="PSUM") as psA_pool:
                        # Wq streamed in column quarters (2 heads each) so the
                        # first q-proj matmuls can start while later quarters
                        # are still in flight
                        wq_t = [wqA_pool.tile([128, HD1], BF, name=f"wq{kd}")
                                for kd in range(KD)]
                        QW = HD1 // 4
                        for q4 in range(4):
                            for kd in range(KD):
                                nc.sync.dma_start(
                                    out=wq_t[kd][:, q4 * QW:(q4 + 1) * QW],
                                    in_=wq_d[kd * 128:(kd + 1) * 128,
                                             q4 * QW:(q4 + 1) * QW])

                        for j in range(JA):
                            jj = slice(j * NA, (j + 1) * NA)
                            if j == 0:
                                hs_ch = hs0_ch
                            else:
                                hs_ch = []
                                for kd in range(KD):
                                    t = hsA_pool.tile([128, NA], BF, tag="hsA",
                                                      name=f"hsA_{j}_{kd}")
                                    nc.sync.dma_start(
                                        out=t[:],
                                        in_=hsT_d[kd * 128:(kd + 1) * 128, jj])
                                    hs_ch.append(t)
                            cq = ropeq_pool.tile([Dh, NA], BF, tag="cq",
                                                 name=f"cq{j}")
                            sq = ropeq_pool.tile([Dh, NA], BF, tag="sq",
                                                 name=f"sq{j}")
                            nc.sync.dma_start(out=cq[:], in_=cosq_d[:, jj])
                            nc.sync.dma_start(out=sq[:], in_=sinq_d[:, jj])
                            for h in range(H):
                                ps = psA_pool.tile([128, NA], F32, tag="psA",
                                                   name=f"psQ{j}_{h}")
                                for kd in range(KD):
                                    nc.tensor.matmul(
                                        ps[:], wq_t[kd][:, h * Dh:(h + 1) * Dh],
                                        hs_ch[kd][:],
                                        start=(kd == 0), stop=(kd == KD - 1))
                                t1 = tmpA_pool.tile([128, NA], F32, tag="t1",
                                                    name=f"t1q{j}_{h}")
                                t2 = tmpA_pool.tile([128, NA], F32, tag="t2",
                                                    name=f"t2q{j}_{h}")
                                nc.vector.tensor_mul(t1[:], ps[:], cq[:])
                                nc.vector.tensor_mul(t2[0:half, :],
                                                     ps[half:Dh, :],
                                                     sq[0:half, :])
                                nc.vector.tensor_mul(t2[half:Dh, :],
                                                     ps[0:half, :],
                                                     sq[half:Dh, :])
                                nc.vector.tensor_add(qT_t[h][:, jj],
                                                     t1[:], t2[:])

                # ========== outer pools for attention + o_proj ==========
                with tc.tile_pool(name="ats", bufs=1) as ats_pool, \
                     tc.tile_pool(name="wo1", bufs=1) as wo1_pool:
                    ats_t = [[ats_pool.tile([Dh, NC], BF, name=f"ats{h}_{jc}")
                              for jc in range(JC)] for h in range(H)]
                    wo1_t = []
                    for h in range(H):
                        w = wo1_pool.tile([128, DH2], BF, name=f"wo1_{h}")
                        nc.sync.dma_start(
                            out=w[:], in_=wo_d[h * 128:(h + 1) * 128, 0:DH2])
                        wo1_t.append(w)

                    with tc.tile_pool(name="kT", bufs=1) as kT_pool, \
                         tc.tile_pool(name="v", bufs=1) as v_pool:
                        kT_t = [kT_pool.tile([Dh, S], BF, name=f"kT{h}")
                                for h in range(H)]
                        v_t = [v_pool.tile([128, HD1], BF, name=f"v{i}")
                               for i in range(SK)]

                        # ========== Phase B: kT (RoPE, bf16 path) + v =====
                        with nc.named_scope("kv"), \
                             tc.tile_pool(name="latB", bufs=1) as latB_pool:
                            lat_t = [latB_pool.tile([128, S], BF,
                                                    name=f"latB{ld}")
                                     for ld in range(LD)]
                            for ld in range(LD):
                                for r in range(4):
                                    nc.sync.dma_start(
                                        out=lat_t[ld][:, r * SQ:(r + 1) * SQ],
                                        in_=latg_d[r * L + ld * 128:
                                                   r * L + (ld + 1) * 128, :])
                            with tc.tile_pool(name="wkv", bufs=1) as wkv_pool, \
                                 tc.tile_pool(name="ropek", bufs=1) as ropek_pool, \
                                 tc.tile_pool(name="tmpB", bufs=6) as tmpB_pool, \
                                 tc.tile_pool(name="psB", bufs=4, space="PSUM") as psB_pool, \
                                 tc.tile_pool(name="psV", bufs=4, space="PSUM") as psV_pool:
                                wk_t = []
                                wv_t = []
                                for ld in range(LD):
                                    w = wkv_pool.tile([128, HD1], BF,
                                                      name=f"wk{ld}")
                                    nc.sync.dma_start(
                                        out=w[:],
                                        in_=wk_d[ld * 128:(ld + 1) * 128, :])
                                    wk_t.append(w)
                                    w = wkv_pool.tile([128, HD1], BF,
                                                      name=f"wv{ld}")
                                    nc.sync.dma_start(
                                        out=w[:],
                                        in_=wv_d[ld * 128:(ld + 1) * 128, :])
                                    wv_t.append(w)
                                cosk_sb = ropek_pool.tile([Dh, S], BF)
                                sink_sb = ropek_pool.tile([Dh, S], BF)
                                nc.sync.dma_start(out=cosk_sb[:], in_=cosk_d[:])
                                nc.sync.dma_start(out=sink_sb[:], in_=sink_d[:])

                                # v-groups interleaved 1:1 into the kT loop:
                                # the kT RoPE is DVE-bound, the v matmuls are
                                # pure PE + ScalarE, so together they balance
                                vjobs = [(i, cch) for i in range(SK)
                                         for cch in range(2)]
                                for h in range(H):
                                    for j in range(S // NA):
                                        jj = slice(j * NA, (j + 1) * NA)
                                        ps = psB_pool.tile(
                                            [128, NA], F32, tag="psB",
                                            name=f"psK{h}_{j}")
                                        for ld in range(LD):
                                            nc.tensor.matmul(
                                                ps[:],
                                                wk_t[ld][:, h * Dh:(h + 1) * Dh],
                                                lat_t[ld][:, jj],
                                                start=(ld == 0),
                                                stop=(ld == LD - 1))
                                        # v group as PE filler
                                        vi, cch = vjobs[h * (S // NA) + j]
                                        cc = slice(cch * NA, (cch + 1) * NA)
                                        psv = psV_pool.tile(
                                            [128, NA], F32, tag="psV",
                                            name=f"psV{vi}_{cch}")
                                        for ld in range(LD):
                                            nc.tensor.matmul(
                                                psv[:],
                                                lat_t[ld][:,
                                                          vi * 128:(vi + 1) * 128],
                                                wv_t[ld][:, cc],
                                                start=(ld == 0),
                                                stop=(ld == LD - 1))
                                        nc.scalar.copy(v_t[vi][:, cc], psv[:])
                                        # kT RoPE. The cos-multiply runs on
                                        # GpSimd (via a bf16 staging copy on
                                        # the otherwise-idle ScalarE) -- the
                                        # DVE is phase B's bottleneck. The
                                        # rotate-half muls stay on DVE reading
                                        # PSUM (mixed PSUM+SBUF operands may
                                        # differ in base partition; SBUF+SBUF
                                        # may not).
                                        kb = tmpB_pool.tile([128, NA], BF,
                                                            tag="tb",
                                                            name=f"kb{h}_{j}")
                                        nc.scalar.copy(kb[:], ps[:])
                                        t1 = tmpB_pool.tile([128, NA], BF,
                                                            tag="tb",
                                                            name=f"t1k{h}_{j}")
                                        t2 = tmpB_pool.tile([128, NA], BF,
                                                            tag="tb",
                                                            name=f"t2k{h}_{j}")
                                        nc.gpsimd.tensor_mul(
                                            t1[:], kb[:], cosk_sb[:, jj])
                                        nc.vector.tensor_mul(
                                            t2[0:half, :], ps[half:Dh, :],
                                            sink_sb[0:half, jj])
                                        nc.vector.tensor_mul(
                                            t2[half:Dh, :], ps[0:half, :],
                                            sink_sb[half:Dh, jj])
                                        nc.vector.tensor_add(
                                            kT_t[h][:, jj], t1[:], t2[:])

                        # ===== Phase C: attention, o_proj half interleaved ==
                        with nc.named_scope("attn"), \
                             tc.tile_pool(name="ET", bufs=6) as et_pool, \
                             tc.tile_pool(name="tree", bufs=8) as tree_pool, \
                             tc.tile_pool(name="rinv", bufs=2) as rinv_pool, \
                             tc.tile_pool(name="outst", bufs=4) as outst_pool, \
                             tc.tile_pool(name="psBT", bufs=2, space="PSUM") as psBT_pool, \
                             tc.tile_pool(name="psPV", bufs=2, space="PSUM") as psPV_pool, \
                             tc.tile_pool(name="psD", bufs=1, space="PSUM") as psD_pool, \
                             tc.tile_pool(name="psR", bufs=1, space="PSUM") as psR_pool:

                            d_pending = []

                            def emit_d_group(wo_tiles, col0):
                                jcp, tl, ncol = d_pending.pop(0)
                                t_global = jcp * (NC // 128) + tl
                                rows = slice(t_global * 128,
                                             (t_global + 1) * 128)
                                cc = slice(ncol * 512, (ncol + 1) * 512)
                                ps = psD_pool.tile(
                                    [128, 512], F32, tag="psD",
                                    name=f"psD{jcp}_{tl}_{ncol}")
                                for h2 in range(H):
                                    nc.tensor.matmul(
                                        ps[:],
                                        ats_t[h2][jcp][:,
                                                       tl * 128:(tl + 1) * 128],
                                        wo_tiles[h2][:, cc],
                                        start=(h2 == 0), stop=(h2 == H - 1))
                                st = outst_pool.tile(
                                    [128, 512], F32, tag="outst",
                                    name=f"st{jcp}_{tl}_{ncol}")
                                nc.vector.tensor_copy(st[:], ps[:])
                                nc.sync.dma_start(
                                    out=out_d[rows,
                                              col0 + ncol * 512:
                                              col0 + (ncol + 1) * 512],
                                    in_=st[:])

                            for jc in range(JC):
                                jj = slice(jc * NC, (jc + 1) * NC)
                                for h in range(H):
                                    ets = []
                                    sums = []   # pairwise tree over et tiles
                                    pv = psPV_pool.tile(
                                        [Dh, NC], F32, tag="pv",
                                        name=f"pv{h}_{jc}")
                                    for g in range(SK // 2):
                                        bt = psBT_pool.tile(
                                            [128, 1024], F32, tag="bt",
                                            name=f"bt{h}_{jc}_{g}")
                                        for p in range(2):
                                            i = g * 2 + p
                                            nc.tensor.matmul(
                                                bt[:, p * 512:(p + 1) * 512],
                                                kT_t[h][:, i * 128:(i + 1) * 128],
                                                qT_t[h][:, jj],
                                                start=True, stop=True)
                                        et = et_pool.tile(
                                            [128, 1024], BF, tag="ET",
                                            name=f"et{h}_{jc}_{g}")
                                        nc.scalar.activation(et[:], bt[:], Exp)
                                        ets.append(et)
                                        if g % 2 == 1:
                                            w = tree_pool.tile(
                                                [128, 1024], BF, tag="tr",
                                                name=f"s{h}_{jc}_{g}")
                                            nc.vector.tensor_add(
                                                w[:], ets[g - 1][:], ets[g][:])
                                            sums.append(w)
                                        if g in (2, 5) and d_pending:
                                            emit_d_group(wo1_t, 0)
                                        if g >= 1:
                                            for p in range(2):
                                                i = (g - 1) * 2 + p
                                                nc.tensor.matmul(
                                                    pv[:],
                                                    v_t[i][:, h * Dh:(h + 1) * Dh],
                                                    ets[g - 1][:,
                                                               p * 512:
                                                               (p + 1) * 512],
                                                    start=(i == 0),
                                                    stop=(i == SK - 1))
                                    for p in range(2):
                                        i = (SK // 2 - 1) * 2 + p
                                        nc.tensor.matmul(
                                            pv[:],
                                            v_t[i][:, h * Dh:(h + 1) * Dh],
                                            ets[SK // 2 - 1][:,
                                                             p * 512:
                                                             (p + 1) * 512],
                                            start=(i == 0), stop=(i == SK - 1))
                                    # finish tree: 4 -> 2 -> 1 wide, then fold
                                    while len(sums) > 1:
                                        nxt = []
                                        for a in range(0, len(sums), 2):
                                            w = tree_pool.tile(
                                                [128, 1024], BF, tag="tr",
                                                name=f"w{h}_{jc}_{len(sums)}_{a}")
                                            nc.vector.tensor_add(
                                                w[:], sums[a][:], sums[a + 1][:])
                                            nxt.append(w)
                                        sums = nxt
                                    rfold = tree_pool.tile(
                                        [128, NC], BF, tag="trf", bufs=2,
                                        name=f"rf{h}_{jc}")
                                    nc.vector.tensor_add(
                                        rfold[:], sums[0][:, 0:512],
                                        sums[0][:, 512:1024])
                                    rr = psR_pool.tile([128, NC], F32, tag="rr",
                                                       name=f"rr{h}_{jc}")
                                    nc.tensor.matmul(rr[:], ones_sk[:],
                                                     rfold[:],
                                                     start=True, stop=True)
                                    rbs = rinv_pool.tile([128, NC], F32,
                                                         tag="rbs",
                                                         name=f"rbs{h}_{jc}")
                                    nc.vector.reciprocal_approx_fast(
                                        rbs[:], rr[:])
                                    nc.vector.tensor_mul(
                                        ats_t[h][jc][:], pv[:], rbs[:])
                                # queue o_proj first-half for this jc
                                for tl in range(NC // 128):
                                    for ncol in range(DH2 // 512):
                                        d_pending.append((jc, tl, ncol))
                                # drain backlog from older jc chunks so the
                                # queue never exceeds one jc worth of groups
                                while len(d_pending) > 16:
                                    emit_d_group(wo1_t, 0)
                            with nc.named_scope("oproj1"):
                                while d_pending:
                                    emit_d_group(wo1_t, 0)

                    # ========== Tail: o_proj second half ==========
                    with nc.named_scope("oproj2"), \
                         tc.tile_pool(name="wo2", bufs=1) as wo2_pool, \
                         tc.tile_pool(name="outT", bufs=4) as outT_pool, \
                         tc.tile_pool(name="psT", bufs=4, space="PSUM") as psT_pool:
                        wo2_t = []
                        for h in range(H):
                            w = wo2_pool.tile([128, DH2], BF, name=f"wo2_{h}")
                            nc.sync.dma_start(
                                out=w[:], in_=wo_d[h * 128:(h + 1) * 128,
                                                   DH2:D])
                            wo2_t.append(w)
                        for t in range(S // 128):
                            rows = slice(t * 128, (t + 1) * 128)
                            jcp, tl = divmod(t, NC // 128)
                            for ncol in range(DH2 // 512):
                                cc = slice(ncol * 512, (ncol + 1) * 512)
                                ps = psT_pool.tile([128, 512], F32, tag="psT",
                                                   name=f"psT{t}_{ncol}")
                                for h in range(H):
                                    nc.tensor.matmul(
                                        ps[:],
                                        ats_t[h][jcp][:, tl * 128:(tl + 1) * 128],
                                        wo2_t[h][:, cc],
                                        start=(h == 0), stop=(h == H - 1))
                                st = outT_pool.tile([128, 512], F32, tag="oT",
                                                    name=f"oT{t}_{ncol}")
                                nc.vector.tensor_copy(st[:], ps[:])
                                nc.sync.dma_start(
                                    out=out_d[rows, DH2 + ncol * 512:
                                              DH2 + (ncol + 1) * 512],
                                    in_=st[:])

    nc.compile()
    return nc


def host_inputs(hidden_states, Wq, Wc, Wk, Wv, Wo, S=SEQ, Dh=HEAD_DIM,
                heads_per_core=HEADS_PER_CORE, n_cores=N_CORES):
    """Shard + preprocess full fp32 inputs into per-core bf16 in_maps."""
    scale = 1.0 / np.sqrt(Dh)
    pos = np.arange(S, dtype=np.float32)
    inv_freq = 1.0 / (ROPE_THETA ** (np.arange(0, Dh, 2, dtype=np.float32) / Dh))
    freqs = pos[:, None] * inv_freq
    emb = np.concatenate([freqs, freqs], axis=-1)      # [S, Dh]
    cosT = np.cos(emb).T.copy()                        # [Dh, S]
    sinT = np.sin(emb).T.copy()
    sinT[: Dh // 2] *= -1.0                            # sign baked for the swap trick
    cosq = (cosT * scale).astype(BF_NP)
    sinq = (sinT * scale).astype(BF_NP)
    cosk = cosT.astype(BF_NP)
    sink = sinT.astype(BF_NP)

    hw = heads_per_core * Dh
    in_maps = []
    for c in range(n_cores):
        b, g = divmod(c, 4)
        cols = slice(g * hw, (g + 1) * hw)
        sq = S // 4
        # rotate the query-side seq axis by -g*sq so this core's latent
        # shard is seq chunk 0 (the host un-rotates the output rows)
        rot = -g * sq
        hsT = np.roll(hidden_states[b].T, rot, axis=1)
        in_maps.append({
            "hsT": np.ascontiguousarray(hsT).astype(BF_NP),
            "Wq": np.ascontiguousarray(Wq[:, cols]).astype(BF_NP),
            "Wc": Wc.astype(BF_NP),
            "Wk": np.ascontiguousarray(Wk[:, cols]).astype(BF_NP),
            "Wv": np.ascontiguousarray(Wv[:, cols]).astype(BF_NP),
            "Wo": np.ascontiguousarray(Wo[cols, :]).astype(BF_NP),
            "cosq": np.ascontiguousarray(np.roll(cosq, rot, axis=1)),
            "sinq": np.ascontiguousarray(np.roll(sinq, rot, axis=1)),
            "cosk": cosk, "sink": sink,
        })
    return in_maps


_NC_CACHE = {}


def kernel(hidden_states, Wq, Wc, Wk, Wv, Wo):
    hidden_states = np.asarray(hidden_states, dtype=np.float32)
    if "nc" not in _NC_CACHE:
        _NC_CACHE["nc"] = build_nc()
    nc = _NC_CACHE["nc"]
    in_maps = host_inputs(hidden_states, np.asarray(Wq, np.float32),
                          np.asarray(Wc, np.float32), np.asarray(Wk, np.float32),
                          np.asarray(Wv, np.float32), np.asarray(Wo, np.float32))
    res = run_bass_kernel_spmd(nc, in_maps, list(range(N_CORES))).results
    B, S, D = BATCH, SEQ, D_MODEL
    out = np.zeros((B, S, D), dtype=np.float32)
    for c in range(N_CORES):
        b, g = divmod(c, 4)
        out[b] += np.roll(res[c]["out"], g * (S // 4), axis=0)
    return out
